# revision 10
# baseline (speedup 1.0000x reference)
"""Trainium2 Bass kernel for 16-head RoPE self-attention (S=2048, B=2, D=2048).

Sharding: 8 cores = 2 batches x 4 head-groups (4 heads each). Each core
computes qkv projection for its batch/heads, full attention over its 4
heads, and a partial output projection (its 4-head slice of Wo rows).
Host sums the 4 partial outputs per batch. No cross-core collectives.

Per-core layout choices (all matmul-native, no big transposes):
  - x is fed pre-transposed as xT [D, S]; q,k are produced directly in
    transposed [d, S] per-head layout (d=128 = partition dim).
  - RoPE's rotate_half is a +-1 partition-permutation, applied as a
    128x128 matmul; the elementwise cos/sin combine runs on VectorE.
  - scores^T [j, i] come from a single matmul per (j-tile, i-chunk);
    exp (with the key-position mask bias folded in) runs on ScalarE
    straight out of PSUM, writing bf16.  No max-subtraction: scores are
    ~N(0,1) here, exp cannot overflow.
  - softmax denominator: pairwise DVE adds over the 16 j-tiles, then an
    all-ones f32 matmul broadcasts r across partitions so the division
    is a plain tensor_tensor multiply.
  - PV consumes exp^T directly (v is the stationary operand); output
    projection consumes attn^T directly (Wo rows are stationary),
    producing out^T [D, S] which the host transposes back.
"""

import os
import numpy as np
import ml_dtypes

S, B, D = 2048, 2, 2048
N_HEADS, DQK = 16, 128
ROPE_THETA = 500000.0
N_CORES = 8
CORES_PER_BATCH = 4
NH_LOC = N_HEADS // CORES_PER_BATCH  # 4 heads per core

LAST_RESULT = None  # BassKernelResults of the most recent run (for test.py)

_NC_CACHE = {}


def _build_nc(s, dmodel, nh_loc, d=DQK, sc=512):
    import concourse.tile as tile
    from concourse import bacc, mybir

    bf16 = mybir.dt.bfloat16
    f32 = mybir.dt.float32
    nk = dmodel // 128      # contraction tiles for the projections
    ns = s // 128           # sequence tiles (key side)
    nch = s // sc           # sequence chunks (query side / moving dim)
    hd = nh_loc * d         # local head-dim total (512)
    ne = dmodel // 128      # output-embedding tiles

    nc = bacc.Bacc("TRN2", target_bir_lowering=False, debug=False)
    xT = nc.dram_tensor("xT", [dmodel, s], bf16, kind="ExternalInput")
    wq = nc.dram_tensor("wq", [dmodel, hd], bf16, kind="ExternalInput")
    wk = nc.dram_tensor("wk", [dmodel, hd], bf16, kind="ExternalInput")
    wv = nc.dram_tensor("wv", [dmodel, hd], bf16, kind="ExternalInput")
    wo = nc.dram_tensor("wo", [hd, dmodel], bf16, kind="ExternalInput")
    cosT = nc.dram_tensor("cosT", [d, s], bf16, kind="ExternalInput")
    sinT = nc.dram_tensor("sinT", [d, s], bf16, kind="ExternalInput")
    maskb = nc.dram_tensor("maskb", [128, ns], f32, kind="ExternalInput")
    pmat = nc.dram_tensor("pmat", [d, d], bf16, kind="ExternalInput")
    ones = nc.dram_tensor("ones", [128, 128], bf16, kind="ExternalInput")
    outT = nc.dram_tensor("outT", [dmodel, s], f32, kind="ExternalOutput")

    xT_r = xT.rearrange("(k p) s -> p k s", p=128)
    wq_r = wq.rearrange("(k p) n -> p k n", p=128)
    wk_r = wk.rearrange("(k p) n -> p k n", p=128)
    wv_r = wv.rearrange("(k p) n -> p k n", p=128)
    wo_r = wo.rearrange("(h p) e -> p h e", p=128)
    outT_r = outT.rearrange("(e p) s -> p e s", p=128)

    with tile.TileContext(nc) as tc:
        with tc.tile_pool(name="const", bufs=1) as constp, \
             tc.tile_pool(name="store", bufs=1) as storep:
            qT_sb = storep.tile([128, nh_loc, s], bf16)
            kT_sb = storep.tile([128, nh_loc, s], bf16)
            v_sb = storep.tile([128, ns, hd], bf16)
            attn_sb = storep.tile([128, nh_loc, s], bf16)

            # ---- phase 1: qkv projection + rope -------------------------
            with tc.tile_pool(name="wqkv", bufs=1) as wp, \
                 tc.tile_pool(name="xch", bufs=2) as xp, \
                 tc.tile_pool(name="rope", bufs=3) as rp, \
                 tc.tile_pool(name="ps1", bufs=3, space="PSUM") as ps1, \
                 tc.tile_pool(name="ps1r", bufs=2, space="PSUM") as ps1r:
                # per-k-tile DMAs, ordered so the first q accumulation can
                # start after ~2 tiles have landed instead of ~10 MB.
                wq_sb = wp.tile([128, nk, hd], bf16, tag="wq")
                wk_sb = wp.tile([128, nk, hd], bf16, tag="wk")
                wv_sb = wp.tile([128, nk, hd], bf16, tag="wv")
                xc0 = xp.tile([128, nk, sc], bf16, tag="xc")
                for k in range(nk):
                    nc.sync.dma_start(wq_sb[:, k, :], wq_r[:, k, :])
                    nc.sync.dma_start(xc0[:, k, :], xT_r[:, k, 0:sc])
                cos_sb = constp.tile([128, s], bf16)
                nc.sync.dma_start(cos_sb[:], cosT[:])
                sin_sb = constp.tile([128, s], bf16)
                nc.sync.dma_start(sin_sb[:], sinT[:])
                pmat_sb = constp.tile([128, d], bf16)
                nc.sync.dma_start(pmat_sb[:], pmat[:])
                nc.sync.dma_start(wk_sb[:], wk_r[:])
                nc.sync.dma_start(wv_sb[:], wv_r[:])
                maskb_sb = constp.tile([128, ns], f32)
                nc.sync.dma_start(maskb_sb[:], maskb[:])
                ones_sb = constp.tile([128, 128], bf16)
                nc.sync.dma_start(ones_sb[:], ones[:])
                wo_sb = constp.tile([128, nh_loc, dmodel], bf16)
                nc.sync.dma_start(wo_sb[:], wo_r[:])

                for ch in range(nch):
                    csl = slice(ch * sc, (ch + 1) * sc)
                    if ch == 0:
                        xc = xc0
                    else:
                        xc = xp.tile([128, nk, sc], bf16, tag="xc")
                        nc.sync.dma_start(xc[:], xT_r[:, :, csl])
                    for h in range(nh_loc):
                        hsl = slice(h * d, (h + 1) * d)
                        for w_sb, dstT in ((wq_sb, qT_sb), (wk_sb, kT_sb)):
                            acc = ps1.tile([128, sc], f32, tag="acc")
                            for k in range(nk):
                                nc.tensor.matmul(
                                    acc[:], w_sb[:, k, hsl], xc[:, k, :],
                                    start=(k == 0), stop=(k == nk - 1),
                                )
                            raw = rp.tile([128, sc], bf16, tag="raw")
                            nc.scalar.copy(raw[:], acc[:])
                            rot_ps = ps1r.tile([128, sc], f32, tag="rot")
                            nc.tensor.matmul(rot_ps[:], pmat_sb[:], raw[:],
                                             start=True, stop=True)
                            rot = rp.tile([128, sc], bf16, tag="rot")
                            nc.scalar.copy(rot[:], rot_ps[:])
                            t1 = rp.tile([128, sc], bf16, tag="t1")
                            nc.vector.tensor_mul(t1[:], raw[:], cos_sb[:, csl])
                            t2 = rp.tile([128, sc], bf16, tag="t2")
                            nc.vector.tensor_mul(t2[:], rot[:], sin_sb[:, csl])
                            nc.vector.tensor_add(dstT[:, h, csl], t1[:], t2[:])
                    for stl in range(sc // 128):
                        st = ch * (sc // 128) + stl
                        ssl = slice(stl * 128, (stl + 1) * 128)
                        accv = ps1.tile([128, hd], f32, tag="acc")
                        for k in range(nk):
                            nc.tensor.matmul(
                                accv[:], xc[:, k, ssl], wv_sb[:, k, :],
                                start=(k == 0), stop=(k == nk - 1),
                            )
                        nc.scalar.copy(v_sb[:, st, :], accv[:])

            # ---- phase 2: attention + output projection -----------------
            # query stripes of 2*sc so each ScalarE EXP covers 1024 cols
            # (amortizes the 352-cycle ACTIVATE overhead).
            sw = 2 * sc
            nst = s // sw
            with tc.tile_pool(name="expp", bufs=2) as expp, \
                 tc.tile_pool(name="ph2", bufs=3) as ph2p, \
                 tc.tile_pool(name="qd", bufs=5) as qdp, \
                 tc.tile_pool(name="outp", bufs=4) as outp, \
                 tc.tile_pool(name="ps2s", bufs=2, space="PSUM") as ps2s, \
                 tc.tile_pool(name="ps2p", bufs=2, space="PSUM") as ps2p, \
                 tc.tile_pool(name="ps2o", bufs=2, space="PSUM") as ps2o:
                for ic in range(nst):
                    isl = slice(ic * sw, (ic + 1) * sw)
                    for h in range(nh_loc):
                        hsl = slice(h * d, (h + 1) * d)
                        ex = expp.tile([128, ns, sw], bf16, tag="exp")
                        for jt in range(ns):
                            jsl = slice(jt * 128, (jt + 1) * 128)
                            sc_ps = ps2s.tile([128, sw], f32, tag="scores")
                            for half in range(2):
                                qsl = slice(ic * sw + half * sc,
                                            ic * sw + (half + 1) * sc)
                                nc.tensor.matmul(
                                    sc_ps[:, half * sc:(half + 1) * sc],
                                    kT_sb[:, h, jsl], qT_sb[:, h, qsl],
                                    start=True, stop=True)
                            nc.scalar.activation(
                                ex[:, jt, :], sc_ps[:],
                                mybir.ActivationFunctionType.Exp,
                                bias=maskb_sb[:, jt:jt + 1], scale=1.0,
                            )
                        # denominator: full DVE pair-add tree (bf16), then a
                        # single ones-matmul per half sums the partitions in
                        # f32 PSUM.
                        lvl = []
                        for a in range(ns // 2):
                            p0 = ph2p.tile([128, sw], bf16, tag="pair")
                            nc.vector.tensor_add(p0[:], ex[:, 2 * a, :],
                                                 ex[:, 2 * a + 1, :])
                            lvl.append(p0)
                        while len(lvl) > 1:
                            nxt = []
                            for a in range(0, len(lvl), 2):
                                q4 = qdp.tile([128, sw], bf16, tag="quad")
                                nc.vector.tensor_add(q4[:], lvl[a][:],
                                                     lvl[a + 1][:])
                                nxt.append(q4)
                            lvl = nxt
                        gacc = lvl[0]
                        r_ps = ps2s.tile([128, sw], f32, tag="scores")
                        for half in range(2):
                            fsl = slice(half * sc, (half + 1) * sc)
                            nc.tensor.matmul(r_ps[:, fsl], ones_sb[:],
                                             gacc[:, fsl],
                                             start=True, stop=True)
                        inv = ph2p.tile([128, sw], f32, tag="inv")
                        nc.vector.reciprocal_approx_fast(inv[:], r_ps[:])
                        for half in range(2):
                            fsl = slice(half * sc, (half + 1) * sc)
                            pv_ps = ps2p.tile([128, sc], f32, tag="pv")
                            for jt in range(ns):
                                nc.tensor.matmul(
                                    pv_ps[:], v_sb[:, jt, hsl],
                                    ex[:, jt, fsl],
                                    start=(jt == 0), stop=(jt == ns - 1),
                                )
                            asl = slice(ic * sw + half * sc,
                                        ic * sw + (half + 1) * sc)
                            nc.vector.tensor_mul(attn_sb[:, h, asl],
                                                 pv_ps[:], inv[:, fsl])
                    for et in range(ne):
                        esl = slice(et * 128, (et + 1) * 128)
                        for half in range(2):
                            osl = slice(ic * sw + half * sc,
                                        ic * sw + (half + 1) * sc)
                            op_ps = ps2o.tile([128, sc], f32, tag="oproj")
                            for ht in range(nh_loc):
                                nc.tensor.matmul(
                                    op_ps[:], wo_sb[:, ht, esl],
                                    attn_sb[:, ht, osl],
                                    start=(ht == 0), stop=(ht == nh_loc - 1),
                                )
                            ot = outp.tile([128, sc], f32, tag="ot")
                            nc.vector.tensor_copy(ot[:], op_ps[:])
                            nc.sync.dma_start(outT_r[:, et, osl], ot[:])

    nc.compile()
    return nc


def _get_nc(s=S, dmodel=D, nh_loc=NH_LOC):
    key = (s, dmodel, nh_loc)
    if key not in _NC_CACHE:
        _NC_CACHE[key] = _build_nc(s, dmodel, nh_loc)
    return _NC_CACHE[key]


def _rope_tables(s, d, dtype=np.float32):
    inv_freq = 1.0 / (ROPE_THETA ** (np.arange(0, d, 2, dtype=np.float64) / d))
    pos = np.arange(s, dtype=np.float64)
    freqs = pos[:, None] * inv_freq[None, :]            # [s, d/2]
    emb = np.concatenate([freqs, freqs], axis=-1)       # [s, d]
    return np.cos(emb).astype(dtype), np.sin(emb).astype(dtype)


def _pmat(d):
    p = np.zeros((d, d), dtype=np.float32)
    h = d // 2
    for m in range(h):
        p[m + h, m] = -1.0
    for m in range(h, d):
        p[m - h, m] = 1.0
    return p


def make_in_maps(hidden_states, sequence_mask, Wqkv, Wo,
                 s=S, b=B, dmodel=D, nh_tot=N_HEADS, nh_loc=NH_LOC, d=DQK):
    bf = ml_dtypes.bfloat16
    cos, sin = _rope_tables(s, d)
    cosT = np.ascontiguousarray(cos.T).astype(bf)       # [d, s]
    sinT = np.ascontiguousarray(sin.T).astype(bf)
    pm = _pmat(d).astype(bf)
    ones = np.ones((128, 128), dtype=bf)
    scale = 1.0 / np.sqrt(np.float32(d))

    in_maps = []
    cores_per_batch = N_CORES // b
    for c in range(N_CORES):
        bi = c // cores_per_batch
        g = c % cores_per_batch
        h0 = g * nh_loc
        hsl = slice(h0 * d, (h0 + nh_loc) * d)
        xb = hidden_states[:, bi, :]                    # [s, dmodel]
        xT = np.ascontiguousarray(xb.T).astype(bf)      # [dmodel, s]
        wq = (Wqkv[:, 0 * nh_tot * d:1 * nh_tot * d][:, hsl] * scale).astype(bf)
        wk = np.ascontiguousarray(
            Wqkv[:, 1 * nh_tot * d:2 * nh_tot * d][:, hsl]).astype(bf)
        wv = np.ascontiguousarray(
            Wqkv[:, 2 * nh_tot * d:3 * nh_tot * d][:, hsl]).astype(bf)
        wo = np.ascontiguousarray(Wo[hsl, :]).astype(bf)
        bias = np.where(sequence_mask[bi] == 0, -1e30, 0.0).astype(np.float32)
        maskbT = np.ascontiguousarray(bias.reshape(s // 128, 128).T)  # [128, ns]
        in_maps.append({
            "xT": xT, "wq": wq, "wk": wk, "wv": wv, "wo": wo,
            "cosT": cosT, "sinT": sinT, "maskb": maskbT,
            "pmat": pm, "ones": ones,
        })
    return in_maps


def kernel(hidden_states, sequence_mask, Wqkv, Wo):
    global LAST_RESULT
    from concourse.bass_utils import run_bass_kernel_spmd

    hidden_states = np.asarray(hidden_states)
    sequence_mask = np.asarray(sequence_mask)
    Wqkv = np.asarray(Wqkv)
    Wo = np.asarray(Wo)

    nc = _get_nc()
    in_maps = make_in_maps(hidden_states, sequence_mask, Wqkv, Wo)
    res = run_bass_kernel_spmd(
        nc, in_maps, list(range(N_CORES)),
        trace=bool(int(os.environ.get("KERNEL_TRACE", "0"))),
    )
    LAST_RESULT = res

    out = np.empty((S, B, D), dtype=np.float32)
    cores_per_batch = N_CORES // B
    for bi in range(B):
        acc = None
        for g in range(cores_per_batch):
            part = res.results[bi * cores_per_batch + g]["outT"]  # [D, S]
            acc = part.copy() if acc is None else acc + part
        out[:, bi, :] = acc.T
    return out


# revision 12
# speedup vs baseline: 1.0823x; 1.0823x over previous
"""Trainium2 Bass kernel for 16-head RoPE self-attention (S=2048, B=2, D=2048).

Sharding: 8 cores = 2 batches x 4 head-groups (4 heads each). Each core
computes qkv projection for its batch/heads, full attention over its 4
heads, and a partial output projection (its 4-head slice of Wo rows).
Host sums the 4 partial outputs per batch. No cross-core collectives.

Per-core layout choices (all matmul-native, no big transposes):
  - x is fed pre-transposed as xT [D, S]; q,k are produced directly in
    transposed [d, S] per-head layout (d=128 = partition dim).
  - RoPE's rotate_half is a +-1 partition-permutation, applied as a
    128x128 matmul; the elementwise cos/sin combine runs on VectorE.
  - scores^T [j, i] come from a single matmul per (j-tile, i-chunk);
    exp (with the key-position mask bias folded in) runs on ScalarE
    straight out of PSUM, writing bf16.  No max-subtraction: scores are
    ~N(0,1) here, exp cannot overflow.
  - softmax denominator: pairwise DVE adds over the 16 j-tiles, then an
    all-ones f32 matmul broadcasts r across partitions so the division
    is a plain tensor_tensor multiply.
  - PV consumes exp^T directly (v is the stationary operand); output
    projection consumes attn^T directly (Wo rows are stationary),
    producing out^T [D, S] which the host transposes back.
"""

import os
import numpy as np
import ml_dtypes

S, B, D = 2048, 2, 2048
N_HEADS, DQK = 16, 128
ROPE_THETA = 500000.0
N_CORES = 8
CORES_PER_BATCH = 4
NH_LOC = N_HEADS // CORES_PER_BATCH  # 4 heads per core

LAST_RESULT = None  # BassKernelResults of the most recent run (for test.py)

_NC_CACHE = {}


def _build_nc(s, dmodel, nh_loc, d=DQK, sc=512):
    import concourse.tile as tile
    from concourse import bacc, mybir

    bf16 = mybir.dt.bfloat16
    f32 = mybir.dt.float32
    nk = dmodel // 128      # contraction tiles for the projections
    ns = s // 128           # sequence tiles (key side)
    nch = s // sc           # sequence chunks (query side / moving dim)
    hd = nh_loc * d         # local head-dim total (512)
    ne = dmodel // 128      # output-embedding tiles

    nc = bacc.Bacc("TRN2", target_bir_lowering=False, debug=False)
    xT = nc.dram_tensor("xT", [dmodel, s], bf16, kind="ExternalInput")
    wq = nc.dram_tensor("wq", [dmodel, hd], bf16, kind="ExternalInput")
    wk = nc.dram_tensor("wk", [dmodel, hd], bf16, kind="ExternalInput")
    wv = nc.dram_tensor("wv", [dmodel, hd], bf16, kind="ExternalInput")
    wo = nc.dram_tensor("wo", [hd, dmodel], bf16, kind="ExternalInput")
    cosT = nc.dram_tensor("cosT", [d, s], bf16, kind="ExternalInput")
    sinT = nc.dram_tensor("sinT", [d, s], bf16, kind="ExternalInput")
    maskb = nc.dram_tensor("maskb", [128, ns], f32, kind="ExternalInput")
    pmat = nc.dram_tensor("pmat", [d, d], bf16, kind="ExternalInput")
    ones = nc.dram_tensor("ones", [128, 128], bf16, kind="ExternalInput")
    outT = nc.dram_tensor("outT", [dmodel, s], f32, kind="ExternalOutput")

    xT_r = xT.rearrange("(k p) s -> p k s", p=128)
    wq_r = wq.rearrange("(k p) n -> p k n", p=128)
    wk_r = wk.rearrange("(k p) n -> p k n", p=128)
    wv_r = wv.rearrange("(k p) n -> p k n", p=128)
    wo_r = wo.rearrange("(h p) e -> p h e", p=128)
    outT_r = outT.rearrange("(e p) s -> p e s", p=128)

    with tile.TileContext(nc) as tc:
        with tc.tile_pool(name="const", bufs=1) as constp, \
             tc.tile_pool(name="store", bufs=1) as storep:
            qT_sb = storep.tile([128, nh_loc, s], bf16)
            kT_sb = storep.tile([128, nh_loc, s], bf16)
            v_sb = storep.tile([128, ns, hd], bf16)
            attn_sb = storep.tile([128, nh_loc, s], bf16)

            # ---- phase 1: qkv projection + rope -------------------------
            with tc.tile_pool(name="wqkv", bufs=1) as wp, \
                 tc.tile_pool(name="xch", bufs=2) as xp, \
                 tc.tile_pool(name="rope", bufs=3) as rp, \
                 tc.tile_pool(name="ps1", bufs=3, space="PSUM") as ps1, \
                 tc.tile_pool(name="ps1r", bufs=2, space="PSUM") as ps1r:
                # per-k-tile DMAs, ordered so the first q accumulation can
                # start after ~2 tiles have landed instead of ~10 MB.
                wq_sb = wp.tile([128, nk, hd], bf16, tag="wq")
                wk_sb = wp.tile([128, nk, hd], bf16, tag="wk")
                wv_sb = wp.tile([128, nk, hd], bf16, tag="wv")
                xc0 = xp.tile([128, nk, sc], bf16, tag="xc")
                for k in range(nk):
                    nc.sync.dma_start(wq_sb[:, k, :], wq_r[:, k, :])
                    nc.sync.dma_start(xc0[:, k, :], xT_r[:, k, 0:sc])
                cos_sb = constp.tile([128, s], bf16)
                nc.sync.dma_start(cos_sb[:], cosT[:])
                sin_sb = constp.tile([128, s], bf16)
                nc.sync.dma_start(sin_sb[:], sinT[:])
                pmat_sb = constp.tile([128, d], bf16)
                nc.sync.dma_start(pmat_sb[:], pmat[:])
                for k in range(nk):
                    nc.sync.dma_start(wk_sb[:, k, :], wk_r[:, k, :])
                for k in range(nk):
                    nc.sync.dma_start(wv_sb[:, k, :], wv_r[:, k, :])
                maskb_sb = constp.tile([128, ns], f32)
                nc.sync.dma_start(maskb_sb[:], maskb[:])
                ones_sb = constp.tile([128, 128], bf16)
                nc.sync.dma_start(ones_sb[:], ones[:])
                wo_sb = constp.tile([128, nh_loc, dmodel], bf16)
                nc.sync.dma_start(wo_sb[:], wo_r[:])

                for ch in range(nch):
                    csl = slice(ch * sc, (ch + 1) * sc)
                    if ch == 0:
                        xc = xc0
                    else:
                        xc = xp.tile([128, nk, sc], bf16, tag="xc")
                        nc.sync.dma_start(xc[:], xT_r[:, :, csl])
                    for h in range(nh_loc):
                        hsl = slice(h * d, (h + 1) * d)
                        for w_sb, dstT in ((wq_sb, qT_sb), (wk_sb, kT_sb)):
                            acc = ps1.tile([128, sc], f32, tag="acc")
                            for k in range(nk):
                                nc.tensor.matmul(
                                    acc[:], w_sb[:, k, hsl], xc[:, k, :],
                                    start=(k == 0), stop=(k == nk - 1),
                                )
                            raw = rp.tile([128, sc], bf16, tag="raw")
                            nc.scalar.copy(raw[:], acc[:])
                            rot_ps = ps1r.tile([128, sc], f32, tag="rot")
                            nc.tensor.matmul(rot_ps[:], pmat_sb[:], raw[:],
                                             start=True, stop=True)
                            rot = rp.tile([128, sc], bf16, tag="rot")
                            nc.scalar.copy(rot[:], rot_ps[:])
                            t1 = rp.tile([128, sc], bf16, tag="t1")
                            nc.vector.tensor_mul(t1[:], raw[:], cos_sb[:, csl])
                            t2 = rp.tile([128, sc], bf16, tag="t2")
                            nc.vector.tensor_mul(t2[:], rot[:], sin_sb[:, csl])
                            nc.vector.tensor_add(dstT[:, h, csl], t1[:], t2[:])
                    for stl in range(sc // 128):
                        st = ch * (sc // 128) + stl
                        ssl = slice(stl * 128, (stl + 1) * 128)
                        accv = ps1.tile([128, hd], f32, tag="acc")
                        for k in range(nk):
                            nc.tensor.matmul(
                                accv[:], xc[:, k, ssl], wv_sb[:, k, :],
                                start=(k == 0), stop=(k == nk - 1),
                            )
                        nc.scalar.copy(v_sb[:, st, :], accv[:])

            # ---- phase 2: attention + output projection -----------------
            # query stripes of 2*sc so each ScalarE EXP covers 1024 cols
            # (amortizes the 352-cycle ACTIVATE overhead).
            sw = 2 * sc
            nst = s // sw
            with tc.tile_pool(name="expp", bufs=2) as expp, \
                 tc.tile_pool(name="ph2", bufs=3) as ph2p, \
                 tc.tile_pool(name="qd", bufs=5) as qdp, \
                 tc.tile_pool(name="outp", bufs=4) as outp, \
                 tc.tile_pool(name="ps2s", bufs=2, space="PSUM") as ps2s, \
                 tc.tile_pool(name="ps2p", bufs=2, space="PSUM") as ps2p, \
                 tc.tile_pool(name="ps2o", bufs=2, space="PSUM") as ps2o:
                for ic in range(nst):
                    isl = slice(ic * sw, (ic + 1) * sw)
                    for h in range(nh_loc):
                        hsl = slice(h * d, (h + 1) * d)
                        ex = expp.tile([128, ns, sw], bf16, tag="exp")
                        for jt in range(ns):
                            jsl = slice(jt * 128, (jt + 1) * 128)
                            sc_ps = ps2s.tile([128, sw], f32, tag="scores")
                            for half in range(2):
                                qsl = slice(ic * sw + half * sc,
                                            ic * sw + (half + 1) * sc)
                                nc.tensor.matmul(
                                    sc_ps[:, half * sc:(half + 1) * sc],
                                    kT_sb[:, h, jsl], qT_sb[:, h, qsl],
                                    start=True, stop=True)
                            nc.scalar.activation(
                                ex[:, jt, :], sc_ps[:],
                                mybir.ActivationFunctionType.Exp,
                                bias=maskb_sb[:, jt:jt + 1], scale=1.0,
                            )
                        # denominator: two DVE pair-add levels (bf16), then
                        # 4 accumulating ones-matmuls per half sum the quads
                        # and the partitions in f32 PSUM.
                        quads = []
                        for a in range(ns // 4):
                            p0 = ph2p.tile([128, sw], bf16, tag="pair")
                            nc.vector.tensor_add(p0[:], ex[:, 4 * a, :],
                                                 ex[:, 4 * a + 1, :])
                            p1 = ph2p.tile([128, sw], bf16, tag="pair2")
                            nc.vector.tensor_add(p1[:], ex[:, 4 * a + 2, :],
                                                 ex[:, 4 * a + 3, :])
                            q4 = qdp.tile([128, sw], bf16, tag="quad")
                            nc.vector.tensor_add(q4[:], p0[:], p1[:])
                            quads.append(q4)
                        r_ps = ps2s.tile([128, sw], f32, tag="scores")
                        for half in range(2):
                            fsl = slice(half * sc, (half + 1) * sc)
                            for qi, q4 in enumerate(quads):
                                nc.tensor.matmul(
                                    r_ps[:, fsl], ones_sb[:], q4[:, fsl],
                                    start=(qi == 0),
                                    stop=(qi == len(quads) - 1))
                        inv = ph2p.tile([128, sw], f32, tag="inv")
                        nc.vector.reciprocal_approx_fast(inv[:], r_ps[:])
                        for half in range(2):
                            fsl = slice(half * sc, (half + 1) * sc)
                            pv_ps = ps2p.tile([128, sc], f32, tag="pv")
                            for jt in range(ns):
                                nc.tensor.matmul(
                                    pv_ps[:], v_sb[:, jt, hsl],
                                    ex[:, jt, fsl],
                                    start=(jt == 0), stop=(jt == ns - 1),
                                )
                            asl = slice(ic * sw + half * sc,
                                        ic * sw + (half + 1) * sc)
                            nc.vector.tensor_mul(attn_sb[:, h, asl],
                                                 pv_ps[:], inv[:, fsl])
                    for et in range(ne):
                        esl = slice(et * 128, (et + 1) * 128)
                        for half in range(2):
                            osl = slice(ic * sw + half * sc,
                                        ic * sw + (half + 1) * sc)
                            op_ps = ps2o.tile([128, sc], f32, tag="oproj")
                            for ht in range(nh_loc):
                                nc.tensor.matmul(
                                    op_ps[:], wo_sb[:, ht, esl],
                                    attn_sb[:, ht, osl],
                                    start=(ht == 0), stop=(ht == nh_loc - 1),
                                )
                            ot = outp.tile([128, sc], f32, tag="ot")
                            nc.vector.tensor_copy(ot[:], op_ps[:])
                            nc.sync.dma_start(outT_r[:, et, osl], ot[:])

    nc.compile()
    return nc


def _get_nc(s=S, dmodel=D, nh_loc=NH_LOC):
    key = (s, dmodel, nh_loc)
    if key not in _NC_CACHE:
        _NC_CACHE[key] = _build_nc(s, dmodel, nh_loc)
    return _NC_CACHE[key]


def _rope_tables(s, d, dtype=np.float32):
    inv_freq = 1.0 / (ROPE_THETA ** (np.arange(0, d, 2, dtype=np.float64) / d))
    pos = np.arange(s, dtype=np.float64)
    freqs = pos[:, None] * inv_freq[None, :]            # [s, d/2]
    emb = np.concatenate([freqs, freqs], axis=-1)       # [s, d]
    return np.cos(emb).astype(dtype), np.sin(emb).astype(dtype)


def _pmat(d):
    p = np.zeros((d, d), dtype=np.float32)
    h = d // 2
    for m in range(h):
        p[m + h, m] = -1.0
    for m in range(h, d):
        p[m - h, m] = 1.0
    return p


def make_in_maps(hidden_states, sequence_mask, Wqkv, Wo,
                 s=S, b=B, dmodel=D, nh_tot=N_HEADS, nh_loc=NH_LOC, d=DQK):
    bf = ml_dtypes.bfloat16
    cos, sin = _rope_tables(s, d)
    cosT = np.ascontiguousarray(cos.T).astype(bf)       # [d, s]
    sinT = np.ascontiguousarray(sin.T).astype(bf)
    pm = _pmat(d).astype(bf)
    ones = np.ones((128, 128), dtype=bf)
    scale = 1.0 / np.sqrt(np.float32(d))

    in_maps = []
    cores_per_batch = N_CORES // b
    for c in range(N_CORES):
        bi = c // cores_per_batch
        g = c % cores_per_batch
        h0 = g * nh_loc
        hsl = slice(h0 * d, (h0 + nh_loc) * d)
        xb = hidden_states[:, bi, :]                    # [s, dmodel]
        xT = np.ascontiguousarray(xb.T).astype(bf)      # [dmodel, s]
        wq = (Wqkv[:, 0 * nh_tot * d:1 * nh_tot * d][:, hsl] * scale).astype(bf)
        wk = np.ascontiguousarray(
            Wqkv[:, 1 * nh_tot * d:2 * nh_tot * d][:, hsl]).astype(bf)
        wv = np.ascontiguousarray(
            Wqkv[:, 2 * nh_tot * d:3 * nh_tot * d][:, hsl]).astype(bf)
        wo = np.ascontiguousarray(Wo[hsl, :]).astype(bf)
        bias = np.where(sequence_mask[bi] == 0, -1e30, 0.0).astype(np.float32)
        maskbT = np.ascontiguousarray(bias.reshape(s // 128, 128).T)  # [128, ns]
        in_maps.append({
            "xT": xT, "wq": wq, "wk": wk, "wv": wv, "wo": wo,
            "cosT": cosT, "sinT": sinT, "maskb": maskbT,
            "pmat": pm, "ones": ones,
        })
    return in_maps


def kernel(hidden_states, sequence_mask, Wqkv, Wo):
    global LAST_RESULT
    from concourse.bass_utils import run_bass_kernel_spmd

    hidden_states = np.asarray(hidden_states)
    sequence_mask = np.asarray(sequence_mask)
    Wqkv = np.asarray(Wqkv)
    Wo = np.asarray(Wo)

    nc = _get_nc()
    in_maps = make_in_maps(hidden_states, sequence_mask, Wqkv, Wo)
    res = run_bass_kernel_spmd(
        nc, in_maps, list(range(N_CORES)),
        trace=bool(int(os.environ.get("KERNEL_TRACE", "0"))),
    )
    LAST_RESULT = res

    out = np.empty((S, B, D), dtype=np.float32)
    cores_per_batch = N_CORES // B
    for bi in range(B):
        acc = None
        for g in range(cores_per_batch):
            part = res.results[bi * cores_per_batch + g]["outT"]  # [D, S]
            acc = part.copy() if acc is None else acc + part
        out[:, bi, :] = acc.T
    return out
